# revision 23
# baseline (speedup 1.0000x reference)
"""CRF loss (nn_ConditionalRandomField) Trainium2 Bass kernel, v4.

Segmented-probe design, 64 segments of 8 steps: 63 packed chains run in 8
independent lockstep groups of 8 slots (group free = 512 = 8 chains x 64
batch; last group 7 slots / 448). Tick 0 is folded into a host-precomputed
initial state s1. Per group-tick: one 128x512 matmul (block-diag [G ; G^T]
bf16 weights) into the group's PSUM bank, then a 512-wide multiply by the
host-packed exp-emission stream, routed per a rotated schedule across
three engines:
  A: DVE mul direct from PSUM (fp8 E),
  C: Act copy PSUM->SBUF bf16 + DVE 2x all-bf16 mul (bf16 E),
  D: Act copy + GPSIMD mul (fp8 E).
22 A / 18 C / 16 D over 56 group-ticks balances DVE/Act/Pool at ~20.5us;
8 concurrent groups hide each chain's ~1-2.1us serial latency. E streams
are consolidated into one fp8 and one bf16 DRAM tensor in (tick, group)
order, fully preloaded into SBUF (no refills); s1 ships as two bf16
tensors (first 2 groups first, for a fast start).

The host runs the rank-1 segment-product telescope over the returned
boundary states in float64 and assembles the loss with the exact gold-path
numerator (total rel err ~5e-4 vs float64 oracle, budget 2e-2).

Assumes harness shapes: B=512, L=512, T=64, mask all ones.
"""
import os
import sys
import numpy as np
import ml_dtypes

for p in ["/root/.axon_site", "/root/.axon_site/_ro/trn_rl_repo",
          "/root/.axon_site/_ro/pypackages"]:
    if p not in sys.path:
        sys.path.insert(0, p)

import concourse.bacc as bacc
import concourse.bass as bass
import concourse.tile as tile
import concourse.mybir as mybir
from concourse.bass_utils import run_bass_kernel_spmd

F32 = mybir.dt.float32
FP16 = mybir.dt.float16
BF16 = mybir.dt.bfloat16
FP8 = mybir.dt.float8e4
ALU = mybir.AluOpType
ACTF = mybir.ActivationFunctionType

NT = 62
START, STOP = 62, 63
B, L, T = 512, 512, 64
NB = 64                  # batch per core
LSEG = 8                 # ticks per chain (tick 0 on host)
SSEG = 64                # segments
NCH = 63                 # chains
NG = 8                   # groups
NTK = LSEG - 1           # device ticks
SE = 62.0                # E-stream scale (fp8 range centering)
GS_LN = 1.0              # weights scaled by e^{-GS_LN}
LAM = 1.0 / 64           # fwd-half s1 scale (fp8/bf16 range centering)
LAM2 = 2.0 ** -27        # global s1 scale so fp16 outs stay in range

GW = [512] * 7 + [448]   # group free widths
GCH = [list(range(8 * g, min(8 * g + 8, NCH))) for g in range(NG)]

# Routes: A = DVE mul direct from PSUM (fp8 E); C = Act copy to SBUF bf16 +
# DVE 2x mul (bf16 E); D = Act copy + GPSIMD mul (fp8 E). Groups 0-6 rotate
# the base pattern (3A 2C 2D per group, per tick); group 7 adds C-heavy.
_BASE = "ACDACAD"
ROUTES = ["".join(_BASE[(k - g) % 7] for k in range(NTK)) for g in range(7)]
ROUTES.append("DADCDCA")
RANK = {"A": 0, "C": 1, "D": 2}

# final-tick out staging slots: groups ordered by final-route speed (A,C,D)
_FIN = sorted(range(NG), key=lambda g: (RANK[ROUTES[g][NTK - 1]], g))
OUT_SLOT = {g: i for i, g in enumerate(_FIN)}
OUT_CHUNKS = [(0, 2), (2, 2), (4, 2), (6, 1), (7, 1)]

# s1 ship order: g0,g1 first (fast start), then tick-0 F-route users, then
# tick-0 C-route users
S1ORD = [0, 1] + sorted(range(2, NG), key=lambda g: ROUTES[g][0] == "C")
S1POS = {g: i for i, g in enumerate(S1ORD)}

# stream layouts: (tick, group) sorted lists
FSEQ = [(k, g) for k in range(NTK) for g in range(NG) if ROUTES[g][k] != "C"]
BSEQ = [(k, g) for k in range(NTK) for g in range(NG) if ROUTES[g][k] == "C"]
FIDX = {kg: i for i, kg in enumerate(FSEQ)}
BIDX = {kg: i for i, kg in enumerate(BSEQ)}

NPBF16 = ml_dtypes.bfloat16
NPFP8 = ml_dtypes.float8_e4m3

_cached = {}


def _kernel_body(tc, nc, aps):
    import contextlib
    ctx = contextlib.ExitStack()
    consts = ctx.enter_context(tc.tile_pool(name="consts", bufs=1))
    spools = [ctx.enter_context(tc.tile_pool(name=f"s{g}", bufs=2))
              for g in range(NG)]
    vpools = [ctx.enter_context(tc.tile_pool(name=f"v{g}", bufs=1, space="PSUM"))
              for g in range(NG)]
    cpools = [ctx.enter_context(tc.tile_pool(name=f"cp{g}", bufs=2))
              for g in range(NG)]
    epool = ctx.enter_context(tc.tile_pool(name="e", bufs=1))

    wt = consts.tile([128, 128], BF16)
    nc.sync.dma_start(out=wt, in_=aps["wt"])

    # initial states (host-precomputed s1): three DMAs for a fast start
    s1a = consts.tile([128, 2, 512], BF16, tag="s1a")
    nc.sync.dma_start(out=s1a, in_=aps["s1a"])
    states = [None] * NG
    for g in range(2):
        states[g] = s1a[:, S1POS[g], :GW[g]]

    # E streams: fully preloaded, one chunk per tick, (tick, group) order;
    # tick-0 chunks land before the bulk of s1 so the pipeline starts fast
    etile = [[None] * NTK for _ in range(NG)]

    def load_tick(k):
        for stream, seq, dt_, ap in (("F", FSEQ, FP8, aps["ef"]),
                                     ("B", BSEQ, BF16, aps["eb"])):
            items = [(kk, g) for (kk, g) in seq if kk == k]
            if not items:
                continue
            j0 = FIDX[items[0]] if stream == "F" else BIDX[items[0]]
            et = epool.tile([128, len(items), 512], dt_, tag=f"e{stream}{k}",
                            name=f"e{stream}{k}")
            nc.sync.dma_start(out=et, in_=ap[:, j0:j0 + len(items), :])
            for j, (kk, g) in enumerate(items):
                etile[g][k] = et[:, j, :GW[g]]

    load_tick(0)
    s1b = consts.tile([128, 3, 512], BF16, tag="s1b")
    nc.sync.dma_start(out=s1b, in_=aps["s1b"])
    for g in S1ORD[2:5]:
        states[g] = s1b[:, S1POS[g] - 2, :GW[g]]
    s1c = consts.tile([128, 3, 512], BF16, tag="s1c")
    nc.sync.dma_start(out=s1c, in_=aps["s1c"])
    for g in S1ORD[5:]:
        states[g] = s1c[:, S1POS[g] - 5, :GW[g]]
    for k in range(1, NTK):
        load_tick(k)

    # final-tick muls write a contiguous staging tile, DMA'd out in chunks
    # of 2 groups ordered by expected finish (A first, D last)
    stage = consts.tile([128, NG, 512], FP16, tag="stage")

    for k in range(NTK):
        if k == 0:
            mm_order = list(S1ORD)
        else:
            mm_order = sorted(range(NG), key=lambda g: RANK[ROUTES[g][k - 1]])
        last = k == NTK - 1

        def s2_of(g):
            if last:
                return stage[:, OUT_SLOT[g], :GW[g]]
            st = spools[g].tile([128, GW[g]], BF16, tag=f"st{g}",
                                name=f"st{g}_{k}")
            return st

        vts = [None] * NG
        for g in mm_order:
            v = vpools[g].tile([128, GW[g]], F32, tag=f"ps{g}")
            nc.tensor.matmul(v, wt, states[g], start=True, stop=True)
            vts[g] = v
        cps = [None] * NG
        for g in mm_order:
            if ROUTES[g][k] in "CD":
                cp = cpools[g].tile([128, GW[g]], BF16, tag=f"c{g}",
                                    name=f"c{g}_{k}")
                nc.scalar.activation(out=cp, in_=vts[g], func=ACTF.Copy)
                cps[g] = cp
        s2s = [None] * NG
        for g in mm_order:
            if ROUTES[g][k] == "A":
                s2s[g] = s2 = s2_of(g)
                nc.vector.tensor_mul(s2, vts[g], etile[g][k])
        for g in mm_order:
            if ROUTES[g][k] == "C":
                s2s[g] = s2 = s2_of(g)
                nc.vector.tensor_mul(s2, cps[g], etile[g][k])
        for g in mm_order:
            if ROUTES[g][k] == "D":
                s2s[g] = s2 = s2_of(g)
                nc.gpsimd.tensor_mul(s2, cps[g], etile[g][k])
        for g in range(NG):
            states[g] = s2s[g]

    out_eng = [nc.scalar, nc.sync, nc.scalar, nc.sync, nc.scalar]
    for i, (j0, n) in enumerate(OUT_CHUNKS):
        out_eng[i].dma_start(out=aps[f"out{i}"], in_=stage[:, j0:j0 + n, :])
    ctx.close()


def _build_module():
    nc = bacc.Bacc("TRN2", target_bir_lowering=False, debug=False,
                   num_devices=8)
    aps = {
        "wt": nc.dram_tensor("wt", [128, 128], BF16, kind="ExternalInput").ap(),
        "s1a": nc.dram_tensor("s1a", [128, 2, 512], BF16,
                              kind="ExternalInput").ap(),
        "s1b": nc.dram_tensor("s1b", [128, 3, 512], BF16,
                              kind="ExternalInput").ap(),
        "s1c": nc.dram_tensor("s1c", [128, 3, 512], BF16,
                              kind="ExternalInput").ap(),
        "ef": nc.dram_tensor("ef", [128, len(FSEQ), 512], FP8,
                             kind="ExternalInput").ap(),
        "eb": nc.dram_tensor("eb", [128, len(BSEQ), 512], BF16,
                             kind="ExternalInput").ap(),
    }
    for i, (j0, n) in enumerate(OUT_CHUNKS):
        aps[f"out{i}"] = nc.dram_tensor(f"out{i}", [128, n, 512], FP16,
                                        kind="ExternalOutput").ap()
    with tile.TileContext(nc) as tc:
        _kernel_body(tc, nc, aps)
    nc.compile()
    return nc


def _host_prep(inputs, transitions):
    trans = np.asarray(transitions, np.float64)
    G = np.exp(trans[:NT, :NT])
    Gs = G * np.exp(-GS_LN)
    g_r = Gs.sum(axis=1)
    D = np.exp(trans[STOP, :NT])

    wt = np.zeros((128, 128), NPBF16)
    wt[0:NT, 0:NT] = Gs.T          # out[0:62] = Gs @ s
    wt[64:64 + NT, 64:64 + NT] = Gs  # out[64:126] = Gs^T @ s

    x = np.asarray(inputs, np.float32).reshape(8, NB, L, T)
    E = np.exp(x[:, :, :, :NT].astype(np.float64))        # [8, NB, L, 62]
    csum = E.sum(axis=3)                                  # [8, NB, L]
    En = E / csum[:, :, :, None]

    a0 = np.exp(trans[:NT, START])[None, None, :] * E[:, :, 0, :]
    ln_a0 = np.log(a0.sum(axis=2))                        # [8, NB]
    a0 = a0 / a0.sum(axis=2, keepdims=True)
    w0 = En[:, :, L - 1, :] * D[None, None, :]
    ln_w0 = np.log(w0.sum(axis=2))
    w0 = w0 / w0.sum(axis=2, keepdims=True)

    # positions: chain ch device-tick k (abs tick k+1) reads En[8ch+k+1]
    # fwd; bwd seg s(ch) (ch>=1 -> ch, ch=0 -> 63) reads En[8s+7-(k+1)].
    # s1 (abs tick 0): fwd = LAM*g_r*SE*En[8ch] (chain 0: a0), bwd =
    # SE*En[8s+7] (chain 0: w0).
    ch_idx = np.arange(NCH)
    k_idx = np.arange(1, LSEG)
    fpos = 8 * ch_idx[:, None] + k_idx[None, :]           # [63, 7]
    sseg = np.where(ch_idx >= 1, ch_idx, 63)
    bpos = 8 * sseg[:, None] + 7 - k_idx[None, :]         # [63, 7]

    in_maps = []
    for c in range(8):
        En_c = En[c]                                      # [64, 512, 62]
        s1f = LAM * g_r[:, None, None] * (SE * En_c[:, 8 * ch_idx, :]
                                          ).transpose(2, 1, 0)
        s1b_ = (SE * En_c[:, 8 * sseg + 7, :]).transpose(2, 1, 0)
        s1f[:, 0, :] = a0[c].T
        s1b_[:, 0, :] = w0[c].T
        s1f *= LAM2
        s1b_ *= LAM2
        fw = (SE * En_c[:, fpos, :]).transpose(2, 3, 1, 0)  # [62,7k,63ch,64b]
        bw = (SE * En_c[:, bpos, :]).transpose(2, 3, 1, 0)

        s1 = np.zeros((128, NG, 512), NPBF16)
        for g in range(NG):
            chs = GCH[g]
            fr = GW[g]
            s1[0:NT, S1POS[g], :fr] = s1f[:, chs, :].reshape(NT, fr)
            s1[64:64 + NT, S1POS[g], :fr] = s1b_[:, chs, :].reshape(NT, fr)
        ef = np.zeros((128, len(FSEQ), 512), NPFP8)
        eb = np.zeros((128, len(BSEQ), 512), NPBF16)
        for g in range(NG):
            chs = GCH[g]
            fr = GW[g]
            ft = fw[:, :, chs, :].reshape(NT, NTK, fr)
            bt = bw[:, :, chs, :].reshape(NT, NTK, fr)
            for k in range(NTK):
                if ROUTES[g][k] == "C":
                    eb[0:NT, BIDX[(k, g)], :fr] = ft[:, k, :]
                    eb[64:64 + NT, BIDX[(k, g)], :fr] = bt[:, k, :]
                else:
                    ef[0:NT, FIDX[(k, g)], :fr] = ft[:, k, :]
                    ef[64:64 + NT, FIDX[(k, g)], :fr] = bt[:, k, :]
        m = {"wt": wt, "s1a": s1[:, 0:2, :], "s1b": s1[:, 2:5, :],
             "s1c": s1[:, 5:NG, :], "ef": ef, "eb": eb}
        in_maps.append(m)

    book = dict(Gs=Gs, ln_a0=ln_a0, ln_w0=ln_w0,
                lncsum=np.log(csum[:, :, 1:]).sum(axis=2))
    return in_maps, book


def _stitch_core(res_c, book, c):
    Gs = book["Gs"]
    y = {}
    wst = {}
    for g in range(NG):
        slot = OUT_SLOT[g]
        ci = next(i for i, (j0, n) in enumerate(OUT_CHUNKS)
                  if j0 <= slot < j0 + n)
        st = res_c[f"out{ci}"][:, slot - OUT_CHUNKS[ci][0], :].astype(np.float64)
        for j, ch in enumerate(GCH[g]):
            y[ch] = st[0:NT, j * NB:(j + 1) * NB]
            wst[ch] = st[64:64 + NT, j * NB:(j + 1) * NB]
    z = {ch: Gs.T @ wst[ch] for ch in wst}
    alpha, beta = y[0], z[0]

    def lndot(a, b):
        return np.log(np.einsum("ib,ib->b", a, b))

    last = NCH - 1
    lnZ = lndot(beta, y[last])
    for i in range(1, last):
        lnZ += lndot(z[i + 1], y[i])
    lnZ += lndot(z[1], alpha)
    for i in range(1, NCH):
        lnZ -= np.log(z[i].sum(axis=0))
    n_se = LSEG * (SSEG - 2) + (LSEG - 1) * 2
    lnZ += n_se * (GS_LN - np.log(SE)) + GS_LN
    lnZ += -(NCH - 1) * np.log(LAM)     # fwd-half s1 scaling, chains >= 1
    lnZ += -(NCH + 1) * np.log(LAM2)    # global s1 scaling (fp16 outs)
    lnZ += book["ln_a0"][c] + book["ln_w0"][c] + book["lncsum"][c]
    return lnZ


def _numerator(inputs, tags, mask, transitions):
    x = np.asarray(inputs, np.float64)
    tg = np.asarray(tags, np.int64)
    mk = np.asarray(mask, np.float64)
    tr = np.asarray(transitions, np.float64)
    Bb, Ll = tg.shape
    score = tr[tg[:, 0], START].copy()
    prev_t, next_t = tg[:, :-1], tg[:, 1:]
    trans_sc = tr[next_t, prev_t]
    bidx = np.arange(Bb)[:, None]
    tidx = np.arange(Ll - 1)[None, :]
    emit_sc = x[bidx, tidx, prev_t]
    score += (trans_sc * mk[:, 1:] + emit_sc * mk[:, :-1]).sum(axis=1)
    last_emit = x[np.arange(Bb), Ll - 1, tg[:, -1]]
    score += tr[STOP, tg[:, -1]] + last_emit * mk[:, -1]
    return score


def kernel(inputs, tags, mask, transitions):
    assert np.all(np.asarray(mask) == 1), "kernel assumes mask of all ones"
    if "nc" not in _cached:
        _cached["nc"] = _build_module()
    nc = _cached["nc"]
    in_maps, book = _host_prep(inputs, transitions)
    res = run_bass_kernel_spmd(nc, in_maps, core_ids=list(range(8)),
                               trace=bool(int(os.environ.get("K_TRACE", "0"))))
    _cached["last"] = res
    score = _numerator(inputs, tags, mask, transitions)
    total = float(score.sum())
    for c in range(8):
        total -= float(_stitch_core(res.results[c], book, c).sum())
    return np.float32(total)
